# Initial kernel scaffold
#
"""Trainium2 Bass kernel for LocalSparseAttention.

Problem (hardcoded): B=2, S=2048, D=1024, H=16, HD=64, WINDOW=128 (band
|i-j| <= 64), fp32 I/O.

Sharding: 8 cores = 2 batches x 4 head-groups (4 heads each). Each core:
  - qk projection into transposed layout [512, 2048] (head-pair packed)
  - v projection into natural layout, 19 (possibly 64-shifted) seq chunks
  - banded attention: per 128-query tile, 256-key window, additive mask
    applied via identity-matmul into PSUM, exp on ACT, AV + softmax
    denominator via ones-augmented v, normalization via PE broadcast
  - output projection -> fp16 partial [2048, 1024]
Host: fp16 casts/transposes in, sum of 4 partials per batch + fused bias
(b_out + b_v @ w_out) out.

All matmuls run in fp16 (1 cycle/row on PE, ~3e-4 rel err) with fp32 PSUM
accumulation; softmax exp input stays fp32.
"""
import sys

if "/opt/trn_rl_repo" not in sys.path:
    sys.path.insert(0, "/opt/trn_rl_repo")

import numpy as np

import concourse.bass as bass
import concourse.mybir as mybir
import concourse.tile as tile
from concourse import bacc
from concourse.bass_utils import run_bass_kernel_spmd

B, S, D, H, HD = 2, 2048, 1024, 16, 64
SCALE = HD**-0.5
C_SUB = 4.0  # subtracted from all scores via the mask; cancels in softmax
MASK_NEG = -30000.0

F16 = mybir.dt.float16
F32 = mybir.dt.float32
F32R = mybir.dt.float32r

# 19 key/value chunk offsets: 15 shifted (128c+64) + aligned 0,128,1792,1920
OFFS = [128 * c + 64 for c in range(15)] + [0, 128, 1792, 1920]


def _chunk_pair(i):
    if i == 0:
        return 15, 16
    if i == 15:
        return 17, 18
    return i - 1, i


def _mask_variant(i):
    return 0 if i == 0 else (2 if i == 15 else 1)


def _build_pair_masks():
    # variant 0: (first, interior) — c4=0 pair 0
    # variant 1: (interior, interior)
    # variant 2: (interior, last)  — c4=3 pair 1
    m = _build_masks()  # [128, 3(first/int/last), 2(half), 128]
    mp = np.zeros((128, 3, 2, 2, 128), np.float16)
    mp[:, 0, 0] = m[:, 0]
    mp[:, 0, 1] = m[:, 1]
    mp[:, 1, 0] = m[:, 1]
    mp[:, 1, 1] = m[:, 1]
    mp[:, 2, 0] = m[:, 1]
    mp[:, 2, 1] = m[:, 2]
    return mp


def _build_masks():
    kp = np.arange(128)[:, None]
    p = np.arange(128)[None, :]
    masks = np.zeros((128, 3, 2, 128), np.float16)
    for v, shift in enumerate([0, 64, 128]):
        for half in (0, 1):
            w = 128 * half + kp
            valid = np.abs(p + shift - w) <= 64
            masks[:, v, half, :] = valid.astype(np.float16)
    return masks


def _build_program(stage=5, nc4=4, nhp=2):
    nc = bacc.Bacc("TRN2", debug=False, num_devices=8)

    xT_d = nc.dram_tensor("xT", [D, S], F16, kind="ExternalInput").ap()
    wqk_d = nc.dram_tensor("wqk", [D, 512], F16, kind="ExternalInput").ap()
    wv_d = nc.dram_tensor("wv", [D, 256], F16, kind="ExternalInput").ap()
    wout_d = nc.dram_tensor("wout", [256, D], F16, kind="ExternalInput").ap()
    bqk_d = nc.dram_tensor("bqk", [128, 4], F32, kind="ExternalInput").ap()
    masks_d = nc.dram_tensor("masks", [128, 3, 2, 2, 128], F16,
                             kind="ExternalInput").ap()
    out_d = nc.dram_tensor("out", [S, D], F16, kind="ExternalOutput").ap()

    with tile.TileContext(nc) as tc:
        with (
            tc.tile_pool(name="const", bufs=1) as cpool,
            tc.tile_pool(name="work", bufs=2) as wpool,
            tc.tile_pool(name="expp", bufs=10) as epool,
            tc.tile_pool(name="ysb", bufs=3) as ypool,
            tc.tile_pool(name="ps512", bufs=2, space="PSUM") as ps512,
            tc.tile_pool(name="psv", bufs=2, space="PSUM") as psv,
            tc.tile_pool(name="pssc", bufs=2, space="PSUM") as pssc,
            tc.tile_pool(name="psav", bufs=2, space="PSUM") as psav,
        ):
            # ---- persistent SBUF tensors ----
            xT_sb = cpool.tile([128, 8, S], F16, tag="xT")
            wqk_sb = cpool.tile([128, 8, 512], F16, tag="wqk")
            wv_sb = cpool.tile([128, 8, 256], F16, tag="wv")
            wout_sb = cpool.tile([128, 2, D], F16, tag="wout")
            bqk_sb = cpool.tile([128, 4], F32, tag="bqk")
            masks_sb = cpool.tile([128, 3, 2, 2, 128], F16, tag="masks")
            qk_sb = cpool.tile([128, 4, S], F16, tag="qk")
            v_sb = cpool.tile([128, 19, 4, 65], F16, tag="v")
            aoT_sb = cpool.tile([128, 2, S], F16, tag="aoT")
            ones_sb = cpool.tile([128, 64], F16, tag="ones")
            onescol_sb = cpool.tile([128, 1], F16, tag="onescol")
            negc_sb = cpool.tile([128, 1], F32, tag="negc")

            # ---- input DMAs, split per k-chunk so compute starts early ----
            xT_r = xT_d.rearrange("(ko kp) s -> kp ko s", kp=128)
            wqk_r = wqk_d.rearrange("(ko kp) n -> kp ko n", kp=128)
            wv_r = wv_d.rearrange("(ko kp) n -> kp ko n", kp=128)
            for kt in range(8):
                nc.sync.dma_start(out=wqk_sb[:, kt], in_=wqk_r[:, kt])
                nc.sync.dma_start(out=wv_sb[:, kt], in_=wv_r[:, kt])
                nc.sync.dma_start(out=xT_sb[:, kt], in_=xT_r[:, kt])
            nc.sync.dma_start(out=bqk_sb[:], in_=bqk_d)
            nc.sync.dma_start(out=masks_sb[:], in_=masks_d)
            nc.sync.dma_start(
                out=wout_sb[:],
                in_=wout_d.rearrange("(t p) n -> p t n", p=128),
            )
            nc.vector.memset(ones_sb[:], 1.0)
            nc.vector.memset(onescol_sb[:], 1.0)
            nc.vector.memset(negc_sb[:], -C_SUB)
            nc.vector.memset(v_sb[:, :, :, 64:65], 1.0)

            # ---- PE warmup: ~7us of dummy matmuls on zeroed SBUF so the
            # HAM clock-gate reaches 8/8 before the first real matmul (which
            # waits ~9us for input DMA) ----
            wsrc = cpool.tile([128, 512], F16, tag="wsrc")
            wdst = cpool.tile([128, 512], F16, tag="wdst")
            nc.vector.memset(wsrc[:], 0.0)
            wps = ps512.tile([128, 512], F32, tag="ps512")
            for w in range(20):
                nc.tensor.matmul(
                    out=wps[:],
                    lhsT=wsrc[:, 0:128],
                    rhs=wsrc[:],
                    start=(w == 0),
                    stop=(w == 19),
                )
            nc.scalar.copy(out=wdst[:], in_=wps[:])


            # ---- emission helpers (B work interleaved into C keeps the
            # PE array duty high so HAM stays at full clock) ----
            def emit_qk_chunk(ns, pools=None):
                # all 4 m-tiles of q/k projection for seq chunk ns.
                # `pools` round-robins PSUM tags so many accumulation
                # groups stay open while xT chunks stream in.
                for m in range(4):
                    scale = SCALE if m < 2 else 1.0
                    if pools is None:
                        ps = ps512.tile([128, 512], F32, tag="ps512")
                    else:
                        pool, tg = pools[m % len(pools)]
                        ps = pool.tile([128, 512], F32, tag=tg)
                    for kt in range(8):
                        nc.tensor.matmul(
                            out=ps[:],
                            lhsT=wqk_sb[:, kt, m * 128:(m + 1) * 128],
                            rhs=xT_sb[:, kt, ns * 512:(ns + 1) * 512],
                            start=(kt == 0),
                            stop=(kt == 7),
                        )
                    nc.scalar.activation(
                        out=qk_sb[:, m, ns * 512:(ns + 1) * 512],
                        in_=ps[:],
                        func=mybir.ActivationFunctionType.Identity,
                        bias=bqk_sb[:, m:m + 1],
                        scale=scale,
                    )

            def emit_v_chunk(c, pool_tag=None):
                off = OFFS[c]
                if pool_tag is None:
                    ps = psv.tile([128, 256], F32, tag="psv")
                else:
                    pool, tg = pool_tag
                    ps = pool.tile([128, 256], F32, tag=tg)
                for kt in range(8):
                    nc.tensor.matmul(
                        out=ps[:],
                        lhsT=xT_sb[:, kt, off:off + 128],
                        rhs=wv_sb[:, kt, :],
                        start=(kt == 0),
                        stop=(kt == 7),
                    )
                nc.scalar.copy(
                    out=v_sb[:, c, :, 0:64],
                    in_=ps[:].rearrange("p (h d) -> p h d", h=4),
                )

            def emit_scores(c4, hp):
                # scores + exp for both heads of the pair
                ex_big0 = epool.tile([128, 4, 2, 128], F16, tag="exp")
                ex_big1 = epool.tile([128, 4, 2, 128], F16, tag="exp")
                ex_big = {0: ex_big0, 1: ex_big1}
                for pair in range(2):
                    if c4 == 0 and pair == 0:
                        pv = 0
                    elif c4 == 3 and pair == 1:
                        pv = 2
                    else:
                        pv = 1
                    sc_h0 = pssc.tile([128, 2, 2, 128], F32, tag="pssc")
                    sc_h1 = pssc.tile([128, 2, 2, 128], F32, tag="pssc")
                    scs = {0: sc_h0, 1: sc_h1}
                    for iw in range(2):
                        ii = pair * 2 + iw
                        i = c4 * 4 + ii
                        cA, cB = _chunk_pair(i)
                        for hh in range(2):
                            po = hh * 64
                            for half, cc in enumerate((cA, cB)):
                                off = OFFS[cc]
                                nc.tensor.matmul(
                                    out=scs[hh][:, iw, half, :],
                                    lhsT=qk_sb[po:po + 64, 2 + hp,
                                               off:off + 128],
                                    rhs=qk_sb[po:po + 64, hp,
                                              i * 128:(i + 1) * 128],
                                    start=(iw == 0 and half == 0),
                                    stop=(iw == 1 and half == 1),
                                )
                    for hh in range(2):
                        # exp(score - C) on ACT (one op per ii-pair),
                        # band-zeroing via 0/1 mask multiply on DVE
                        sl = slice(pair * 2, pair * 2 + 2)
                        nc.scalar.activation(
                            out=ex_big[hh][:, sl],
                            in_=scs[hh][:],
                            func=mybir.ActivationFunctionType.Exp,
                            bias=negc_sb[:],
                        )
                        nc.vector.tensor_mul(
                            out=ex_big[hh][:, sl],
                            in0=ex_big[hh][:, sl],
                            in1=masks_sb[:, pv],
                        )
                return ex_big

            def emit_av_norm(c4, hp, ex_big):
                av = {}
                for hh in range(2):
                    h = 2 * hp + hh
                    avt = psav.tile([128, 4, 128], F32, tag="psav")
                    av[hh] = avt
                    mwidth = 65 if hh == 0 else 64
                    outsl = slice(0, 65) if hh == 0 else slice(64, 128)
                    for ii in range(4):
                        cA, cB = _chunk_pair(c4 * 4 + ii)
                        for half, cc in enumerate((cA, cB)):
                            nc.tensor.matmul(
                                out=avt[outsl, ii, :],
                                lhsT=v_sb[:, cc, h, 0:mwidth],
                                rhs=ex_big[hh][:, ii, half, :],
                                start=(ii == 0 and half == 0),
                                stop=(ii == 3 and half == 1),
                            )
                    if hh == 1:
                        # odd-head denominators: two strided N=512 matmuls
                        # (partition-0 group, disjoint from the data rows)
                        for half in range(2):
                            nc.tensor.matmul(
                                out=avt[0:1, :, :],
                                lhsT=onescol_sb[:],
                                rhs=ex_big[hh][:, :, half, :],
                                start=(half == 0),
                                stop=(half == 1),
                            )

                # normalization: denoms -> SBUF f16, PE broadcast,
                # approx-reciprocal on the broadcast, multiply
                den = wpool.tile([65, 512], F16, tag="den")
                nc.scalar.copy(
                    out=den[64:65, :],
                    in_=av[0][64:65, :, :].rearrange("p a b -> p (a b)"),
                )
                nc.scalar.copy(
                    out=den[0:1, :],
                    in_=av[1][0:1, :, :].rearrange("p a b -> p (a b)"),
                )
                bc = ps512.tile([128, 512], F32, tag="ps512")
                nc.tensor.matmul(
                    out=bc[0:64, :], lhsT=ones_sb[64:65, :],
                    rhs=den[64:65, :], start=True, stop=True,
                )
                nc.tensor.matmul(
                    out=bc[64:128, :], lhsT=ones_sb[0:1, :],
                    rhs=den[0:1, :], start=True, stop=True,
                )
                bcs = wpool.tile([128, 512], F32, tag="bcs")
                nc.vector.reciprocal_approx_fast(out=bcs[:], in_=bc[:])
                sl = slice(c4 * 512, (c4 + 1) * 512)
                nc.vector.tensor_mul(
                    out=aoT_sb[0:64, hp, sl],
                    in0=av[0][0:64, :, :].rearrange("p a b -> p (a b)"),
                    in1=bcs[0:64, :],
                )
                nc.vector.tensor_mul(
                    out=aoT_sb[64:128, hp, sl],
                    in0=av[1][64:128, :, :].rearrange("p a b -> p (a b)"),
                    in1=bcs[64:128, :],
                )

            def emit_outproj_st(st):
                if True:
                    for nn in range(2):
                        ps = ps512.tile([128, 512], F32, tag="ps512")
                        for hp2 in range(2):
                            nc.tensor.matmul(
                                out=ps[:],
                                lhsT=aoT_sb[:, hp2, st * 128:(st + 1) * 128],
                                rhs=wout_sb[:, hp2,
                                            nn * 512:(nn + 1) * 512],
                                start=(hp2 == 0),
                                stop=(hp2 == 1),
                            )
                        ysb = ypool.tile([128, 512], F16, tag="ysb")
                        if (st * 2 + nn) % 2 == 0:
                            nc.scalar.copy(out=ysb[:], in_=ps[:])
                        else:
                            nc.vector.tensor_copy(out=ysb[:], in_=ps[:])
                        nc.sync.dma_start(
                            out=out_d[st * 128:(st + 1) * 128,
                                      nn * 512:(nn + 1) * 512],
                            in_=ysb[:],
                        )

            # ---- emission schedule: prologue B; per pair-block the PE
            # stream is [scores | filler (prev-block outproj) | AV | B-next
            # chunks] so the PE has independent work while ACT runs exp ----
            rrp = [(ps512, "ps512"), (pssc, "pssc"), (psav, "psav"),
                   (ps512, "ps512")]
            emit_qk_chunk(0, pools=rrp)
            emit_qk_chunk(1, pools=[(pssc, "pssc"), (psav, "psav"),
                                    (ps512, "ps512"), (pssc, "pssc")])
            vrr = [(psv, "psv"), (psav, "psav"), (psv, "psv"),
                   (pssc, "pssc"), (psv, "psv"), (psv, "psv")]
            for c, pt in zip((15, 16, 0, 1, 2, 3), vrr):
                emit_v_chunk(c, pool_tag=pt)

            mid = {
                (1, 0): [0, 1], (1, 1): [2, 3],
                (2, 0): [4, 5], (2, 1): [6, 7],
                (3, 0): [8, 9], (3, 1): [10, 11],
            }
            after = {
                (0, 0): ("qk", [2]), (0, 1): ("v", [4, 5, 6, 7]),
                (1, 0): ("qk", [3]), (1, 1): ("v", [8, 9, 10, 11]),
                (2, 0): ("v", [12, 13]), (2, 1): ("v", [14, 17, 18]),
                (3, 0): ("v", []), (3, 1): ("v", []),
            }
            for c4 in range(4):
                for hp in range(2):
                    ex_big = emit_scores(c4, hp)
                    for st in mid.get((c4, hp), []):
                        emit_outproj_st(st)
                    emit_av_norm(c4, hp, ex_big)
                    kind, items = after[(c4, hp)]
                    for it in items:
                        if kind == "qk":
                            emit_qk_chunk(it)
                        else:
                            emit_v_chunk(it)
            for st in range(12, 16):
                emit_outproj_st(st)

    nc.compile()
    return nc


_NC = None


def _get_program():
    global _NC
    if _NC is None:
        _NC = _build_program()
    return _NC


def _make_in_maps(x, w_qkv, b_qkv, w_out):
    masks = _build_pair_masks()

    in_maps = []
    for c in range(8):
        b, hg = divmod(c, 4)
        cq = 256 * hg
        wqk = np.concatenate(
            [w_qkv[:, cq:cq + 256], w_qkv[:, 1024 + cq:1024 + cq + 256]],
            axis=1,
        ).astype(np.float16)
        bqk = np.empty((128, 4), np.float32)
        bqk[:, 0] = b_qkv[cq:cq + 128] * SCALE
        bqk[:, 1] = b_qkv[cq + 128:cq + 256] * SCALE
        bqk[:, 2] = b_qkv[1024 + cq:1024 + cq + 128]
        bqk[:, 3] = b_qkv[1024 + cq + 128:1024 + cq + 256]
        in_maps.append({
            "xT": np.ascontiguousarray(x[b].T).astype(np.float16),
            "wqk": wqk,
            "wv": w_qkv[:, 2048 + cq:2048 + cq + 256].astype(np.float16),
            "wout": w_out[cq:cq + 256, :].astype(np.float16),
            "bqk": bqk,
            "masks": masks,
        })
    return in_maps


def kernel(x, w_qkv, b_qkv, w_out, b_out):
    x = np.asarray(x, np.float32)
    w_qkv = np.asarray(w_qkv, np.float32)
    b_qkv = np.asarray(b_qkv, np.float32)
    w_out = np.asarray(w_out, np.float32)
    b_out = np.asarray(b_out, np.float32)

    in_maps = _make_in_maps(x, w_qkv, b_qkv, w_out)
    nc = _get_program()
    res = run_bass_kernel_spmd(nc, in_maps, list(range(8)))

    b_v = b_qkv[2048:]
    bias_all = b_out + b_v @ w_out  # folds the (untracked) v-bias
    y = np.empty((B, S, D), np.float32)
    for b in range(B):
        acc = np.zeros((S, D), np.float32)
        for hg in range(4):
            acc += res.results[4 * b + hg]["out"].astype(np.float32)
        y[b] = acc + bias_all
    return y



# revision 1
# speedup vs baseline: 1.1748x; 1.1748x over previous
"""Trainium2 Bass kernel for LocalSparseAttention.

Problem (hardcoded): B=2, S=2048, D=1024, H=16, HD=64, WINDOW=128 (band
|i-j| <= 64), fp32 I/O.

Sharding: 8 cores = 2 batches x 4 head-groups (4 heads each). Each core:
  - qk projection into transposed layout [512, 2048] (head-pair packed)
  - v projection into natural layout, 19 (possibly 64-shifted) seq chunks
  - banded attention: per 128-query tile, 256-key window, additive mask
    applied via identity-matmul into PSUM, exp on ACT, AV + softmax
    denominator via ones-augmented v, normalization via PE broadcast
  - output projection -> fp16 partial [2048, 1024]
Host: fp16 casts/transposes in, sum of 4 partials per batch + fused bias
(b_out + b_v @ w_out) out.

All matmuls run in fp16 (1 cycle/row on PE, ~3e-4 rel err) with fp32 PSUM
accumulation; softmax exp input stays fp32.
"""
import sys

if "/opt/trn_rl_repo" not in sys.path:
    sys.path.insert(0, "/opt/trn_rl_repo")

import numpy as np

import concourse.bass as bass
import concourse.mybir as mybir
import concourse.tile as tile
from concourse import bacc
from concourse.bass_utils import run_bass_kernel_spmd

B, S, D, H, HD = 2, 2048, 1024, 16, 64
SCALE = HD**-0.5
C_SUB = 4.0  # subtracted from all scores via the mask; cancels in softmax
MASK_NEG = -30000.0

F16 = mybir.dt.float16
F32 = mybir.dt.float32
F32R = mybir.dt.float32r

# 19 key/value chunk offsets: 15 shifted (128c+64) + aligned 0,128,1792,1920
OFFS = [128 * c + 64 for c in range(15)] + [0, 128, 1792, 1920]


def _chunk_pair(i):
    if i == 0:
        return 15, 16
    if i == 15:
        return 17, 18
    return i - 1, i


def _mask_variant(i):
    return 0 if i == 0 else (2 if i == 15 else 1)


def _build_pair_masks():
    # variant 0: (first, interior) — c4=0 pair 0
    # variant 1: (interior, interior)
    # variant 2: (interior, last)  — c4=3 pair 1
    m = _build_masks()  # [128, 3(first/int/last), 2(half), 128]
    mp = np.zeros((128, 3, 2, 2, 128), np.float16)
    mp[:, 0, 0] = m[:, 0]
    mp[:, 0, 1] = m[:, 1]
    mp[:, 1, 0] = m[:, 1]
    mp[:, 1, 1] = m[:, 1]
    mp[:, 2, 0] = m[:, 1]
    mp[:, 2, 1] = m[:, 2]
    return mp


def _build_masks():
    kp = np.arange(128)[:, None]
    p = np.arange(128)[None, :]
    masks = np.zeros((128, 3, 2, 128), np.float16)
    for v, shift in enumerate([0, 64, 128]):
        for half in (0, 1):
            w = 128 * half + kp
            valid = np.abs(p + shift - w) <= 64
            masks[:, v, half, :] = valid.astype(np.float16)
    return masks


def _build_program(stage=5, nc4=4, nhp=2):
    nc = bacc.Bacc("TRN2", debug=False, num_devices=8)

    xT_d = nc.dram_tensor("xT", [D, S], F16, kind="ExternalInput").ap()
    wqk_d = nc.dram_tensor("wqk", [D, 512], F16, kind="ExternalInput").ap()
    wv_d = nc.dram_tensor("wv", [D, 256], F16, kind="ExternalInput").ap()
    wout_d = nc.dram_tensor("wout", [256, D], F16, kind="ExternalInput").ap()
    bqk_d = nc.dram_tensor("bqk", [128, 4], F32, kind="ExternalInput").ap()
    masks_d = nc.dram_tensor("masks", [128, 3, 2, 2, 128], F16,
                             kind="ExternalInput").ap()
    out_d = nc.dram_tensor("out", [S, D], F16, kind="ExternalOutput").ap()

    with tile.TileContext(nc) as tc:
        with (
            tc.tile_pool(name="const", bufs=1) as cpool,
            tc.tile_pool(name="work", bufs=2) as wpool,
            tc.tile_pool(name="expp", bufs=10) as epool,
            tc.tile_pool(name="ysb", bufs=3) as ypool,
            tc.tile_pool(name="ps512", bufs=2, space="PSUM") as ps512,
            tc.tile_pool(name="psv", bufs=2, space="PSUM") as psv,
            tc.tile_pool(name="pssc", bufs=2, space="PSUM") as pssc,
            tc.tile_pool(name="psav", bufs=2, space="PSUM") as psav,
        ):
            # ---- persistent SBUF tensors ----
            xT_sb = cpool.tile([128, 8, S], F16, tag="xT")
            wqk_sb = cpool.tile([128, 8, 512], F16, tag="wqk")
            wv_sb = cpool.tile([128, 8, 256], F16, tag="wv")
            wout_sb = cpool.tile([128, 2, D], F16, tag="wout")
            bqk_sb = cpool.tile([128, 4], F32, tag="bqk")
            masks_sb = cpool.tile([128, 3, 2, 2, 128], F16, tag="masks")
            qk_sb = cpool.tile([128, 4, S], F16, tag="qk")
            v_sb = cpool.tile([128, 19, 4, 65], F16, tag="v")
            aoT_sb = cpool.tile([128, 2, S], F16, tag="aoT")
            ones_sb = cpool.tile([128, 64], F16, tag="ones")
            onescol_sb = cpool.tile([128, 1], F16, tag="onescol")
            negc_sb = cpool.tile([128, 1], F32, tag="negc")

            # ---- input DMAs, split per k-chunk so compute starts early ----
            xT_r = xT_d.rearrange("(ko kp) s -> kp ko s", kp=128)
            wqk_r = wqk_d.rearrange("(ko kp) n -> kp ko n", kp=128)
            wv_r = wv_d.rearrange("(ko kp) n -> kp ko n", kp=128)
            for kt in range(8):
                nc.sync.dma_start(out=wqk_sb[:, kt], in_=wqk_r[:, kt])
                nc.sync.dma_start(out=wv_sb[:, kt], in_=wv_r[:, kt])
                nc.sync.dma_start(out=xT_sb[:, kt], in_=xT_r[:, kt])
            nc.sync.dma_start(out=bqk_sb[:], in_=bqk_d)
            nc.sync.dma_start(out=masks_sb[:], in_=masks_d)
            nc.sync.dma_start(
                out=wout_sb[:],
                in_=wout_d.rearrange("(t p) n -> p t n", p=128),
            )
            nc.vector.memset(ones_sb[:], 1.0)
            nc.vector.memset(onescol_sb[:], 1.0)
            nc.vector.memset(negc_sb[:], -C_SUB)
            nc.vector.memset(v_sb[:, :, :, 64:65], 1.0)

            # ---- PE warmup: ~7us of dummy matmuls on zeroed SBUF so the
            # HAM clock-gate reaches 8/8 before the first real matmul (which
            # waits ~9us for input DMA) ----
            wsrc = cpool.tile([128, 512], F16, tag="wsrc")
            wdst = cpool.tile([128, 512], F16, tag="wdst")
            nc.vector.memset(wsrc[:], 0.0)
            wps = ps512.tile([128, 512], F32, tag="ps512")
            for w in range(20):
                nc.tensor.matmul(
                    out=wps[:],
                    lhsT=wsrc[:, 0:128],
                    rhs=wsrc[:],
                    start=(w == 0),
                    stop=(w == 19),
                )
            nc.scalar.copy(out=wdst[:], in_=wps[:])


            # ---- emission helpers (B work interleaved into C keeps the
            # PE array duty high so HAM stays at full clock) ----
            def emit_qk_chunk(ns, pools=None):
                # all 4 m-tiles of q/k projection for seq chunk ns.
                # `pools` round-robins PSUM tags so many accumulation
                # groups stay open while xT chunks stream in.
                for m in range(4):
                    scale = SCALE if m < 2 else 1.0
                    if pools is None:
                        ps = ps512.tile([128, 512], F32, tag="ps512")
                    else:
                        pool, tg = pools[m % len(pools)]
                        ps = pool.tile([128, 512], F32, tag=tg)
                    for kt in range(8):
                        nc.tensor.matmul(
                            out=ps[:],
                            lhsT=wqk_sb[:, kt, m * 128:(m + 1) * 128],
                            rhs=xT_sb[:, kt, ns * 512:(ns + 1) * 512],
                            start=(kt == 0),
                            stop=(kt == 7),
                        )
                    nc.scalar.activation(
                        out=qk_sb[:, m, ns * 512:(ns + 1) * 512],
                        in_=ps[:],
                        func=mybir.ActivationFunctionType.Identity,
                        bias=bqk_sb[:, m:m + 1],
                        scale=scale,
                    )

            def emit_v_chunk(c, pool_tag=None):
                off = OFFS[c]
                if pool_tag is None:
                    ps = psv.tile([128, 256], F32, tag="psv")
                else:
                    pool, tg = pool_tag
                    ps = pool.tile([128, 256], F32, tag=tg)
                for kt in range(8):
                    nc.tensor.matmul(
                        out=ps[:],
                        lhsT=xT_sb[:, kt, off:off + 128],
                        rhs=wv_sb[:, kt, :],
                        start=(kt == 0),
                        stop=(kt == 7),
                    )
                nc.scalar.copy(
                    out=v_sb[:, c, :, 0:64],
                    in_=ps[:].rearrange("p (h d) -> p h d", h=4),
                )

            def emit_scores(c4, hp):
                # scores + exp for both heads of the pair
                ex_big0 = epool.tile([128, 4, 2, 128], F16, tag="exp")
                ex_big1 = epool.tile([128, 4, 2, 128], F16, tag="exp")
                ex_big = {0: ex_big0, 1: ex_big1}
                for pair in range(2):
                    if c4 == 0 and pair == 0:
                        pv = 0
                    elif c4 == 3 and pair == 1:
                        pv = 2
                    else:
                        pv = 1
                    sc_h0 = pssc.tile([128, 2, 2, 128], F32, tag="pssc")
                    sc_h1 = pssc.tile([128, 2, 2, 128], F32, tag="pssc")
                    scs = {0: sc_h0, 1: sc_h1}
                    for iw in range(2):
                        ii = pair * 2 + iw
                        i = c4 * 4 + ii
                        cA, cB = _chunk_pair(i)
                        for hh in range(2):
                            po = hh * 64
                            for half, cc in enumerate((cA, cB)):
                                off = OFFS[cc]
                                nc.tensor.matmul(
                                    out=scs[hh][:, iw, half, :],
                                    lhsT=qk_sb[po:po + 64, 2 + hp,
                                               off:off + 128],
                                    rhs=qk_sb[po:po + 64, hp,
                                              i * 128:(i + 1) * 128],
                                    start=(iw == 0 and half == 0),
                                    stop=(iw == 1 and half == 1),
                                )
                    for hh in range(2):
                        # exp(score - C) on ACT (one op per ii-pair),
                        # band-zeroing via 0/1 mask multiply on DVE
                        sl = slice(pair * 2, pair * 2 + 2)
                        nc.scalar.activation(
                            out=ex_big[hh][:, sl],
                            in_=scs[hh][:],
                            func=mybir.ActivationFunctionType.Exp,
                            bias=negc_sb[:],
                        )
                        nc.vector.tensor_mul(
                            out=ex_big[hh][:, sl],
                            in0=ex_big[hh][:, sl],
                            in1=masks_sb[:, pv],
                        )
                return ex_big

            def emit_av_norm(c4, hp, ex_big):
                av = {}
                for hh in range(2):
                    h = 2 * hp + hh
                    avt = psav.tile([128, 4, 128], F32, tag="psav")
                    av[hh] = avt
                    mwidth = 65 if hh == 0 else 64
                    outsl = slice(0, 65) if hh == 0 else slice(64, 128)
                    for ii in range(4):
                        cA, cB = _chunk_pair(c4 * 4 + ii)
                        for half, cc in enumerate((cA, cB)):
                            nc.tensor.matmul(
                                out=avt[outsl, ii, :],
                                lhsT=v_sb[:, cc, h, 0:mwidth],
                                rhs=ex_big[hh][:, ii, half, :],
                                start=(ii == 0 and half == 0),
                                stop=(ii == 3 and half == 1),
                            )
                    if hh == 1:
                        # odd-head denominators: two strided N=512 matmuls
                        # (partition-0 group, disjoint from the data rows)
                        for half in range(2):
                            nc.tensor.matmul(
                                out=avt[0:1, :, :],
                                lhsT=onescol_sb[:],
                                rhs=ex_big[hh][:, :, half, :],
                                start=(half == 0),
                                stop=(half == 1),
                            )

                # normalization: denoms -> SBUF f16, PE broadcast,
                # approx-reciprocal on the broadcast, multiply
                den = wpool.tile([65, 512], F16, tag="den")
                nc.scalar.copy(
                    out=den[64:65, :],
                    in_=av[0][64:65, :, :].rearrange("p a b -> p (a b)"),
                )
                nc.scalar.copy(
                    out=den[0:1, :],
                    in_=av[1][0:1, :, :].rearrange("p a b -> p (a b)"),
                )
                bc = ps512.tile([128, 512], F32, tag="ps512")
                nc.tensor.matmul(
                    out=bc[0:64, :], lhsT=ones_sb[64:65, :],
                    rhs=den[64:65, :], start=True, stop=True,
                )
                nc.tensor.matmul(
                    out=bc[64:128, :], lhsT=ones_sb[0:1, :],
                    rhs=den[0:1, :], start=True, stop=True,
                )
                bcs = wpool.tile([128, 512], F32, tag="bcs")
                nc.vector.reciprocal_approx_fast(out=bcs[:], in_=bc[:])
                sl = slice(c4 * 512, (c4 + 1) * 512)
                nc.vector.tensor_mul(
                    out=aoT_sb[0:64, hp, sl],
                    in0=av[0][0:64, :, :].rearrange("p a b -> p (a b)"),
                    in1=bcs[0:64, :],
                )
                nc.vector.tensor_mul(
                    out=aoT_sb[64:128, hp, sl],
                    in0=av[1][64:128, :, :].rearrange("p a b -> p (a b)"),
                    in1=bcs[64:128, :],
                )

            def emit_outproj_st(st):
                if True:
                    for nn in range(2):
                        ps = ps512.tile([128, 512], F32, tag="ps512")
                        for hp2 in range(2):
                            nc.tensor.matmul(
                                out=ps[:],
                                lhsT=aoT_sb[:, hp2, st * 128:(st + 1) * 128],
                                rhs=wout_sb[:, hp2,
                                            nn * 512:(nn + 1) * 512],
                                start=(hp2 == 0),
                                stop=(hp2 == 1),
                            )
                        ysb = ypool.tile([128, 512], F16, tag="ysb")
                        if (st * 2 + nn) % 2 == 0:
                            nc.scalar.copy(out=ysb[:], in_=ps[:])
                        else:
                            nc.vector.tensor_copy(out=ysb[:], in_=ps[:])
                        nc.sync.dma_start(
                            out=out_d[st * 128:(st + 1) * 128,
                                      nn * 512:(nn + 1) * 512],
                            in_=ysb[:],
                        )

            # ---- emission schedule: prologue B; per pair-block the PE
            # stream is [scores | filler (prev-block outproj) | AV | B-next
            # chunks] so the PE has independent work while ACT runs exp ----
            rrp = [(ps512, "ps512"), (pssc, "pssc"), (psav, "psav"),
                   (ps512, "ps512")]
            emit_qk_chunk(0, pools=rrp)
            emit_qk_chunk(1, pools=[(pssc, "pssc"), (psav, "psav"),
                                    (ps512, "ps512"), (pssc, "pssc")])
            vrr = [(psv, "psv"), (psav, "psav"), (psv, "psv"),
                   (pssc, "pssc"), (psv, "psv"), (psv, "psv")]
            for c, pt in zip((15, 16, 0, 1, 2, 3), vrr):
                emit_v_chunk(c, pool_tag=pt)

            mid = {
                (1, 0): [0, 1], (1, 1): [2, 3],
                (2, 0): [4, 5], (2, 1): [6, 7],
                (3, 0): [8, 9], (3, 1): [10, 11],
            }
            after = {
                (0, 0): ("qk", [2]), (0, 1): ("v", [4, 5, 6, 7]),
                (1, 0): ("qk", [3]), (1, 1): ("v", [8, 9, 10, 11]),
                (2, 0): ("v", [12, 13]), (2, 1): ("v", [14, 17, 18]),
                (3, 0): ("v", []), (3, 1): ("v", []),
            }
            for c4 in range(4):
                for hp in range(2):
                    ex_big = emit_scores(c4, hp)
                    for st in mid.get((c4, hp), []):
                        emit_outproj_st(st)
                    emit_av_norm(c4, hp, ex_big)
                    kind, items = after[(c4, hp)]
                    for it in items:
                        if kind == "qk":
                            emit_qk_chunk(it)
                        else:
                            emit_v_chunk(it)
            for st in range(12, 16):
                emit_outproj_st(st)

    nc.compile()
    return nc


_NC = None


def _get_program():
    global _NC
    if _NC is None:
        _NC = _build_program()
    return _NC


def _make_in_maps(x, w_qkv, b_qkv, w_out):
    masks = _build_pair_masks()

    in_maps = []
    for c in range(8):
        b, hg = divmod(c, 4)
        cq = 256 * hg
        wqk = np.concatenate(
            [w_qkv[:, cq:cq + 256], w_qkv[:, 1024 + cq:1024 + cq + 256]],
            axis=1,
        ).astype(np.float16)
        bqk = np.empty((128, 4), np.float32)
        bqk[:, 0] = b_qkv[cq:cq + 128] * SCALE
        bqk[:, 1] = b_qkv[cq + 128:cq + 256] * SCALE
        bqk[:, 2] = b_qkv[1024 + cq:1024 + cq + 128]
        bqk[:, 3] = b_qkv[1024 + cq + 128:1024 + cq + 256]
        in_maps.append({
            "xT": np.ascontiguousarray(x[b].T).astype(np.float16),
            "wqk": wqk,
            "wv": w_qkv[:, 2048 + cq:2048 + cq + 256].astype(np.float16),
            "wout": w_out[cq:cq + 256, :].astype(np.float16),
            "bqk": bqk,
            "masks": masks,
        })
    return in_maps


def kernel(x, w_qkv, b_qkv, w_out, b_out):
    x = np.asarray(x, np.float32)
    w_qkv = np.asarray(w_qkv, np.float32)
    b_qkv = np.asarray(b_qkv, np.float32)
    w_out = np.asarray(w_out, np.float32)
    b_out = np.asarray(b_out, np.float32)

    in_maps = _make_in_maps(x, w_qkv, b_qkv, w_out)
    nc = _get_program()
    res = run_bass_kernel_spmd(nc, in_maps, list(range(8)))

    b_v = b_qkv[2048:]
    bias_all = b_out + b_v @ w_out  # folds the (untracked) v-bias
    y = np.empty((B, S, D), np.float32)
    for b in range(B):
        acc = np.zeros((S, D), np.float32)
        for hg in range(4):
            acc += res.results[4 * b + hg]["out"].astype(np.float32)
        y[b] = acc + bias_all
    return y

